# revision 16
# baseline (speedup 1.0000x reference)
"""Trainium2 Bass kernel for the ConnectedComponentsHead problem.

Sharding: 8 cores, core c owns depth slice d=c for both batches.
Per-core layout: batch 0 on SBUF partitions 0-63, batch 1 on partitions 64-127.

Pipeline (single SPMD launch):
  conv stack (4x conv3x3+bias+relu, float32r matmuls, PE quadrant pairing)
  -> segment sums per label (dma-transpose to [pixel, chan] bf16 tiles,
     one-hot matmul accumulation in PSUM)
  -> AllReduce segment sums across cores
  -> on-core pred_kernel algebra (segment means, mask_kernel linear, BN1d,
     BatchNorm2d-of-logits folded analytically to an affine a*x+c using
     host-precomputed Gram matrices of mask_features)
  -> bmm pred_kernel @ mask_features with fused +c epilogue.

Host (free) work: input slicing, label transposes/bincounts, Gram matrices,
inst_features/pred_kernel outputs from the AllReduced sums, reassembly.
"""

import os
import numpy as np

import concourse.bass as bass
from concourse import bacc, tile, bass_utils, mybir

F32 = mybir.dt.float32
F32R = mybir.dt.float32r
BF16 = mybir.dt.bfloat16
AF = mybir.ActivationFunctionType
OP = mybir.AluOpType
AX = mybir.AxisListType

N_CORES = 8
B, C, D, H, W = 2, 64, 8, 128, 128
L = 101
NLM = 20
KD = 64
NI = 100 + NLM          # 120 instances per batch
V_SLICE = H * W         # 16384 pixels per (b, d) slice
V = D * V_SLICE
EPS = 1e-3
BNV = float(B * 2 * NI * V) / 2.0  # B * N * V = 2*120*131072
INV_BNV = 1.0 / (B * NI * V)

HP = H + 2              # padded row count (130)
WP = W + 2              # padded col count (130)
TAPS = [(dy, dx) for dy in range(3) for dx in range(3)]
ROWS_PER_TILE = 3
N_TILES = (H + ROWS_PER_TILE - 1) // ROWS_PER_TILE  # 43
XB_PASS_T = [0, 11, 22, 33, N_TILES]       # conv3 tile ranges per xb pass
XB_COLS = 11 * ROWS_PER_TILE * W           # 4224 columns per xb pass buffer
CHUNK = 128             # pixels per segment chunk
MF_TILE = 512
N_MF_TILES = V_SLICE // MF_TILE  # 32

_compiled = {}


def _build_nc():
    nc = bacc.Bacc("TRN2", target_bir_lowering=False, debug=False,
                   num_devices=N_CORES)

    # ---- kernel I/O ----
    feats_in = nc.dram_tensor("feats", [B, C, V_SLICE], F32R, kind="ExternalInput")
    mf_in = nc.dram_tensor("mf", [B, C, V_SLICE], F32R, kind="ExternalInput")
    labt_in = nc.dram_tensor("labt", [CHUNK, B * (V_SLICE // CHUNK)], F32,
                             kind="ExternalInput")
    convw_in = nc.dram_tensor("convw", [128, 4 * 9 * 128], F32R, kind="ExternalInput")
    convb_in = nc.dram_tensor("convb", [128, 4], F32, kind="ExternalInput")
    mkwt_in = nc.dram_tensor("mkwt", [C, KD], F32, kind="ExternalInput")
    mkb_in = nc.dram_tensor("mkb", [KD, 1], F32, kind="ExternalInput")
    embt_in = nc.dram_tensor("embt", [C, NLM], F32, kind="ExternalInput")
    nkg_in = nc.dram_tensor("nkg", [KD, 1], F32, kind="ExternalInput")
    nkb_in = nc.dram_tensor("nkb", [KD, 1], F32, kind="ExternalInput")
    nl_in = nc.dram_tensor("nl", [1, 2], F32, kind="ExternalInput")  # [gamma, beta]
    recip_in = nc.dram_tensor("recip", [1, B * L], F32, kind="ExternalInput")
    sb_in = nc.dram_tensor("sb", [C, B], F32, kind="ExternalInput")
    mgram_in = nc.dram_tensor("mgram", [KD, B * KD], F32, kind="ExternalInput")
    ident_in = nc.dram_tensor("ident", [KD, KD], F32, kind="ExternalInput")
    identb_in = nc.dram_tensor("identb", [KD, KD], BF16, kind="ExternalInput")
    ones_in = nc.dram_tensor("ones", [C, 1], F32, kind="ExternalInput")

    x_out = nc.dram_tensor("x_out", [B, C, V_SLICE], F32, kind="ExternalOutput")
    masks_out = nc.dram_tensor("masks_out", [B, 2 * NI // 2, V_SLICE], F32,
                               kind="ExternalOutput")
    seg_out = nc.dram_tensor("seg_out", [C, B * L], F32, kind="ExternalOutput")

    with tile.TileContext(nc) as tc:
        with (
            tc.tile_pool(name="bufs", bufs=1) as bufpool,
            tc.tile_pool(name="const", bufs=1) as cpool,
            tc.tile_pool(name="work", bufs=4) as wpool,
            tc.tile_pool(name="alg", bufs=1) as apool,
            tc.tile_pool(name="cpsum", bufs=2, space="PSUM") as cpsum,
            tc.tile_pool(name="spsum", bufs=1, space="PSUM") as spsum,
            tc.tile_pool(name="dram", bufs=1, space="DRAM") as dram,
        ):
            # ---- persistent SBUF tiles ----
            buf0 = bufpool.tile([128, HP, WP], F32R, tag="buf0")
            buf1 = bufpool.tile([128, HP, WP], F32R, tag="buf1")
            xb0 = bufpool.tile([64, XB_COLS], BF16, tag="xb0")
            xb1 = bufpool.tile([64, XB_COLS], BF16, tag="xb1")

            convw = cpool.tile([128, 4 * 9 * 128], F32R)
            convb = cpool.tile([128, 4], F32)
            labt = cpool.tile([CHUNK, B * (V_SLICE // CHUNK)], F32)
            iota = cpool.tile([CHUNK, L], BF16)
            mkwt = cpool.tile([C, KD], F32)
            mkb = cpool.tile([KD, 1], F32)
            embt = cpool.tile([C, NLM], F32)
            nkg = cpool.tile([KD, 1], F32)
            nkb = cpool.tile([KD, 1], F32)
            nlt = cpool.tile([1, 2], F32)
            recip = cpool.tile([1, B * L], F32)
            sbt = cpool.tile([C, B], F32)
            mgram = cpool.tile([KD, B * KD], F32)
            ident = cpool.tile([KD, KD], F32)
            identb = cpool.tile([KD, KD], BF16)
            ones = cpool.tile([C, 1], F32)

            nc.sync.dma_start(out=convw[:], in_=convw_in[:, :])
            nc.sync.dma_start(out=convb[:], in_=convb_in[:, :])
            nc.sync.dma_start(out=labt[:], in_=labt_in[:, :])
            nc.sync.dma_start(out=mkwt[:], in_=mkwt_in[:, :])
            nc.sync.dma_start(out=mkb[:], in_=mkb_in[:, :])
            nc.sync.dma_start(out=embt[:], in_=embt_in[:, :])
            nc.sync.dma_start(out=nkg[:], in_=nkg_in[:, :])
            nc.sync.dma_start(out=nkb[:], in_=nkb_in[:, :])
            nc.sync.dma_start(out=nlt[:], in_=nl_in[:, :])
            nc.sync.dma_start(out=recip[:], in_=recip_in[:, :])
            nc.sync.dma_start(out=sbt[:], in_=sb_in[:, :])
            nc.sync.dma_start(out=mgram[:], in_=mgram_in[:, :])
            nc.sync.dma_start(out=ident[:], in_=ident_in[:, :])
            nc.sync.dma_start(out=identb[:], in_=identb_in[:, :])
            nc.sync.dma_start(out=ones[:], in_=ones_in[:, :])
            nc.gpsimd.iota(iota[:], pattern=[[1, L]], base=0, channel_multiplier=0,
                           allow_small_or_imprecise_dtypes=True)

            # zero the padding halo of both conv buffers (never rewritten).
            # memset can't emit float32r, so copy from a zeroed fp32 row
            # (f32->f32r copy is a rounding op, which the BIR verifier wants).
            zrow = cpool.tile([128, WP], F32)
            nc.vector.memset(zrow[:], 0.0)
            zcol = zrow.rearrange("p (a b) -> p a b", b=1)
            for buf in (buf0, buf1):
                nc.vector.tensor_copy(buf[:, 0, :], zrow[:])
                nc.vector.tensor_copy(buf[:, HP - 1, :], zrow[:])
                nc.vector.tensor_copy(buf[:, :, 0:1], zcol)
                nc.vector.tensor_copy(buf[:, :, WP - 1:WP], zcol)

            # load conv0 input into the interior of buf0
            for b in range(B):
                nc.sync.dma_start(
                    out=buf0[64 * b:64 * b + 64, 1:H + 1, 1:W + 1],
                    in_=feats_in[b].rearrange("c (h w) -> c h w", w=W))

            # ---- conv stack ----
            bufs = [buf0, buf1]
            for layer in range(4):
                src = bufs[layer % 2]
                dst = bufs[(layer + 1) % 2]
                src_f = src.rearrange("p a b -> p (a b)")
                last = layer == 3
                for t in range(N_TILES):
                    r = ROWS_PER_TILE * t
                    nrows = min(ROWS_PER_TILE, H - r)
                    n = (nrows - 1) * WP + W
                    psA = cpsum.tile([128, ROWS_PER_TILE, WP], F32, tag="psA")
                    psA_f = psA.rearrange("p a b -> p (a b)")
                    for k, (dy, dx) in enumerate(TAPS):
                        off = (r + dy) * WP + dx
                        wcol = (layer * 9 + k) * 128
                        nc.tensor.matmul(psA_f[:, 0:n],
                                         convw[:, wcol:wcol + 128],
                                         src_f[:, off:off + n],
                                         start=(k == 0), stop=(k == 8))
                    if not last:
                        nc.scalar.activation(
                            dst[:, r + 1:r + 1 + nrows, 1:W + 1],
                            psA[:, 0:nrows, 0:W],
                            AF.Relu, bias=convb[:, layer:layer + 1])
                    else:
                        # conv3: write x (f32r dst, DMA'd out via f32 bitcast)
                        # + bf16 copy for the segment path
                        nc.scalar.activation(
                            dst[:, r + 1:r + 1 + nrows, 1:W + 1],
                            psA[:, 0:nrows, 0:W],
                            AF.Relu, bias=convb[:, layer:layer + 1])
                        xpass = next(i for i in range(4)
                                     if XB_PASS_T[i] <= t < XB_PASS_T[i + 1])
                        xcol = (r - ROWS_PER_TILE * XB_PASS_T[xpass]) * W
                        npix = nrows * W
                        nc.vector.tensor_scalar(
                            xb0[:, xcol:xcol + npix],
                            psA[0:64, 0:nrows, 0:W],
                            convb[0:64, layer:layer + 1], 0.0, OP.add, OP.max)
                        nc.vector.tensor_scalar(
                            xb1[:, xcol:xcol + npix],
                            psA[64:128, 0:nrows, 0:W],
                            convb[64:128, layer:layer + 1], 0.0, OP.add, OP.max)
                        # segment matmuls for completed chunks of this pass
                        if t == XB_PASS_T[xpass + 1] - 1:
                            c_lo = XB_PASS_T[xpass] * ROWS_PER_TILE * W // CHUNK
                            c_hi = XB_PASS_T[xpass + 1] * ROWS_PER_TILE * W \
                                // CHUNK
                            c_hi = min(c_hi, V_SLICE // CHUNK)
                            base = XB_PASS_T[xpass] * ROWS_PER_TILE * W
                            for ch in range(c_lo, c_hi):
                                col = ch * CHUNK - base
                                for b in range(B):
                                    xbb = xb0 if b == 0 else xb1
                                    tp = cpsum.tile([CHUNK, C], BF16, tag="psB")
                                    nc.tensor.transpose(
                                        tp[:], xbb[:, col:col + CHUNK],
                                        identb[:])
                                    ft = wpool.tile([CHUNK, C], BF16, tag="ft")
                                    nc.scalar.activation(ft[:], tp[:], AF.Copy)
                                    oh = wpool.tile([CHUNK, L], BF16, tag="oh")
                                    nc.vector.tensor_scalar(
                                        oh[:], iota[:],
                                        labt[:, b * (V_SLICE // CHUNK) + ch:
                                             b * (V_SLICE // CHUNK) + ch + 1],
                                        None, OP.is_equal)
                                    segp = seg_ps0 if b == 0 else seg_ps1
                                    nc.tensor.matmul(
                                        segp[:], ft[:], oh[:],
                                        start=(ch == 0),
                                        stop=(ch == V_SLICE // CHUNK - 1))
                # allocate seg accumulators before conv3's tile loop runs
                if layer == 2:
                    seg_ps0 = spsum.tile([C, L], F32, tag="seg0")
                    seg_ps1 = spsum.tile([C, L], F32, tag="seg1")

            # x output DMA (conv3 wrote into buf0)
            for b in range(B):
                nc.sync.dma_start(
                    out=x_out[b].rearrange("c (h w) -> c h w", w=W),
                    in_=buf0[64 * b:64 * b + 64, 1:H + 1, 1:W + 1].bitcast(F32))

            # ---- segment sums AllReduce ----
            seg_sb = apool.tile([C, B * L], F32)
            nc.scalar.activation(seg_sb[:, 0:L], seg_ps0[:], AF.Copy)
            nc.scalar.activation(seg_sb[:, L:2 * L], seg_ps1[:], AF.Copy)
            cc_in = dram.tile([C, B * L], F32)
            cc_out = dram.tile([C, B * L], F32)
            nc.sync.dma_start(out=cc_in[:], in_=seg_sb[:])
            nc.gpsimd.collective_compute(
                "AllReduce", OP.add,
                replica_groups=[list(range(N_CORES))],
                ins=[cc_in.opt()], outs=[cc_out.opt()])
            sums = apool.tile([C, B * L], F32)
            nc.sync.dma_start(out=sums[:], in_=cc_out[:])
            nc.sync.dma_start(out=seg_out[:, :], in_=sums[:])

            # ---- pred_kernel algebra (identical on every core) ----
            recip_b = apool.tile([C, B * L], F32)
            nc.gpsimd.partition_broadcast(recip_b[:], recip[:])
            means = apool.tile([C, B * L], F32)
            nc.vector.tensor_tensor(means[:], sums[:], recip_b[:], OP.mult)

            instT = apool.tile([C, B * NI], F32)
            nc.vector.tensor_copy(instT[:, 0:100], means[:, 1:101])
            nc.vector.tensor_copy(instT[:, 100:120], embt[:])
            nc.vector.tensor_copy(instT[:, 120:220], means[:, L + 1:L + 101])
            nc.vector.tensor_copy(instT[:, 220:240], embt[:])

            pk_ps = cpsum.tile([KD, B * NI], F32, tag="psA")
            nc.tensor.matmul(pk_ps[:], mkwt[:], instT[:], start=True, stop=True)
            pk_sb = apool.tile([KD, B * NI], F32)
            nc.scalar.activation(pk_sb[:], pk_ps[:], AF.Identity, bias=mkb[:])

            red = apool.tile([KD, 1], F32)
            nc.vector.tensor_reduce(red[:], pk_sb[:], axis=AX.X, op=OP.add)
            mu = apool.tile([KD, 1], F32)
            nc.vector.tensor_scalar(mu[:], red[:], 1.0 / (B * NI), None, OP.mult)
            xc = apool.tile([KD, B * NI], F32)
            nc.vector.tensor_scalar(xc[:], pk_sb[:], mu[:], None, OP.subtract)
            sqs = apool.tile([KD, B * NI], F32)
            ssum = apool.tile([KD, 1], F32)
            nc.scalar.activation(sqs[:], xc[:], AF.Square, accum_out=ssum[:])
            varp = apool.tile([KD, 1], F32)
            nc.vector.tensor_scalar(varp[:], ssum[:], 1.0 / (B * NI), EPS,
                                    OP.mult, OP.add)
            std = apool.tile([KD, 1], F32)
            nc.scalar.activation(std[:], varp[:], AF.Sqrt)
            inv = apool.tile([KD, 1], F32)
            nc.vector.reciprocal(inv[:], std[:])
            sc = apool.tile([KD, 1], F32)
            nc.vector.tensor_tensor(sc[:], inv[:], nkg[:], OP.mult)
            pk_bnT = apool.tile([KD, B * NI], F32)
            nc.vector.tensor_scalar(pk_bnT[:], xc[:], sc[:], nkb[:],
                                    OP.mult, OP.add)

            # mu2 = sum_b s_b . (sum_n pk_bn) / BNV
            tsum = apool.tile([KD, 1], F32)
            pks0 = apool.tile([KD, 1], F32)
            pks1 = apool.tile([KD, 1], F32)
            nc.vector.tensor_reduce(pks0[:], pk_bnT[:, 0:NI], axis=AX.X, op=OP.add)
            nc.vector.tensor_reduce(pks1[:], pk_bnT[:, NI:2 * NI], axis=AX.X,
                                    op=OP.add)
            nc.vector.tensor_tensor(pks0[:], pks0[:], sbt[:, 0:1], OP.mult)
            nc.vector.tensor_tensor(pks1[:], pks1[:], sbt[:, 1:2], OP.mult)
            nc.vector.tensor_tensor(tsum[:], pks0[:], pks1[:], OP.add)
            sc_ps = cpsum.tile([1, 2], F32, tag="psB")
            nc.tensor.matmul(sc_ps[:, 0:1], ones[:], tsum[:], start=True, stop=True)
            mu2 = apool.tile([1, 1], F32)
            nc.scalar.activation(mu2[:], sc_ps[:, 0:1], AF.Copy, scale=INV_BNV)

            # E[x^2] = sum_b <G_b, M_b> / BNV  via PE transpose + Gram matmul
            gsum = apool.tile([KD, 1], F32)
            for b in range(B):
                tp_ps = cpsum.tile([NI, KD], F32, tag="psA")
                nc.tensor.transpose(tp_ps[:], pk_bnT[:, b * NI:(b + 1) * NI],
                                    ident[:])
                pkb = apool.tile([NI, KD], F32, tag=f"pkb{b}")
                nc.scalar.activation(pkb[:], tp_ps[:], AF.Copy)
                g_ps = cpsum.tile([KD, KD], F32, tag="psB")
                nc.tensor.matmul(g_ps[:], pkb[:], pkb[:], start=True, stop=True)
                ghm = apool.tile([KD, KD], F32, tag=f"ghm{b}")
                nc.vector.tensor_tensor(ghm[:], g_ps[:],
                                        mgram[:, b * KD:(b + 1) * KD], OP.mult)
                gr = apool.tile([KD, 1], F32, tag=f"gr{b}")
                nc.vector.tensor_reduce(gr[:], ghm[:], axis=AX.X, op=OP.add)
                if b == 0:
                    nc.vector.tensor_copy(gsum[:], gr[:])
                else:
                    nc.vector.tensor_tensor(gsum[:], gsum[:], gr[:], OP.add)
            e2_ps = cpsum.tile([1, 2], F32, tag="psA")
            nc.tensor.matmul(e2_ps[:, 0:1], ones[:], gsum[:], start=True, stop=True)
            e2 = apool.tile([1, 1], F32)
            nc.scalar.activation(e2[:], e2_ps[:, 0:1], AF.Copy, scale=INV_BNV)

            # a = nl_gamma / sqrt(var2 + eps); c = nl_beta - mu2 * a
            m2sq = apool.tile([1, 1], F32)
            nc.vector.tensor_tensor(m2sq[:], mu2[:], mu2[:], OP.mult)
            var2 = apool.tile([1, 1], F32)
            nc.vector.tensor_tensor(var2[:], e2[:], m2sq[:], OP.subtract)
            nc.vector.tensor_scalar(var2[:], var2[:], EPS, None, OP.add)
            std2 = apool.tile([1, 1], F32)
            nc.scalar.activation(std2[:], var2[:], AF.Sqrt)
            inv2 = apool.tile([1, 1], F32)
            nc.vector.reciprocal(inv2[:], std2[:])
            a_t = apool.tile([1, 1], F32)
            nc.vector.tensor_tensor(a_t[:], inv2[:], nlt[:, 0:1], OP.mult)
            na = apool.tile([1, 1], F32)
            nc.vector.tensor_scalar(na[:], a_t[:], -1.0, None, OP.mult)
            c_t = apool.tile([1, 1], F32)
            nc.vector.scalar_tensor_tensor(c_t[:], mu2[:], na[:], nlt[:, 1:2],
                                           OP.mult, OP.add)
            a_b = apool.tile([128, 1], F32)
            nc.gpsimd.partition_broadcast(a_b[:], a_t[:])
            c_b = apool.tile([128, 1], F32)
            nc.gpsimd.partition_broadcast(c_b[:], c_t[:])

            pk_sT = apool.tile([KD, B * NI], F32R)
            nc.vector.tensor_scalar(pk_sT[:], pk_bnT[:], a_b[0:KD, :], None,
                                    OP.mult)

            # ---- bmm: masks[b] = a*pk_bn[b] @ mf[b] + c ----
            for b in range(B):
                for t in range(N_MF_TILES):
                    mft = wpool.tile([C, MF_TILE], F32R, tag="mf")
                    nc.sync.dma_start(
                        out=mft[:],
                        in_=mf_in[b, :, t * MF_TILE:(t + 1) * MF_TILE])
                    bps = cpsum.tile([NI, MF_TILE], F32, tag="psB")
                    nc.tensor.matmul(bps[:], pk_sT[:, b * NI:(b + 1) * NI],
                                     mft[:], start=True, stop=True)
                    mo = wpool.tile([NI, MF_TILE], F32, tag="mo")
                    nc.scalar.activation(mo[:], bps[:], AF.Identity,
                                         bias=c_b[0:NI, :])
                    nc.sync.dma_start(
                        out=masks_out[b, :, t * MF_TILE:(t + 1) * MF_TILE],
                        in_=mo[:])

    nc.compile()
    return nc


def _get_nc():
    if "nc" not in _compiled:
        _compiled["nc"] = _build_nc()
    return _compiled["nc"]


def kernel(features, mask_features, conv_w, conv_b, embed, mk_w, mk_b,
           nk_gamma, nk_beta, nl_gamma, nl_beta, init_masks):
    features = np.asarray(features, np.float32)
    mask_features = np.asarray(mask_features, np.float32)
    conv_w = np.asarray(conv_w, np.float32)
    conv_b = np.asarray(conv_b, np.float32)
    embed = np.asarray(embed, np.float32)
    mk_w = np.asarray(mk_w, np.float32)
    mk_b = np.asarray(mk_b, np.float32)
    nk_gamma = np.asarray(nk_gamma, np.float32)
    nk_beta = np.asarray(nk_beta, np.float32)
    nl_gamma = np.asarray(nl_gamma, np.float32)
    nl_beta = np.asarray(nl_beta, np.float32)
    init_masks = np.asarray(init_masks, np.int32)

    # ---- host precompute (shared across cores) ----
    # conv weights: per (layer, tap) a [128,128] block-diag lhsT
    # diag(W.T, W.T) so one K=128 matmul computes both batch slices
    convw = np.zeros((128, 4 * 9 * 128), np.float32)
    for i in range(4):
        for k, (dy, dx) in enumerate(TAPS):
            wT = conv_w[i, :, :, 0, dy, dx].T           # [Cin, Cout]
            c0 = (i * 9 + k) * 128
            convw[0:64, c0:c0 + 64] = wT
            convw[64:128, c0 + 64:c0 + 128] = wT
    convb = np.ascontiguousarray(np.tile(conv_b.T, (2, 1)))    # [128, 4]

    counts = np.stack([np.bincount(init_masks[b].reshape(-1), minlength=L)
                       for b in range(B)]).astype(np.float32)   # [B, L]
    recip = (1.0 / np.maximum(counts, 1.0)).reshape(1, B * L)

    mfv = mask_features.reshape(B, C, V)
    sb = np.ascontiguousarray(mfv.sum(axis=2).T)                # [C, B]
    mgram = np.einsum('bkv,bjv->bkj', mfv, mfv,
                      optimize=True).astype(np.float32)
    mgram = np.ascontiguousarray(mgram.transpose(1, 0, 2).reshape(KD, B * KD))

    small = dict(
        convw=convw, convb=convb,
        mkwt=np.ascontiguousarray(mk_w.T),
        mkb=mk_b.reshape(KD, 1).copy(),
        embt=np.ascontiguousarray(embed.T),
        nkg=nk_gamma.reshape(KD, 1).copy(),
        nkb=nk_beta.reshape(KD, 1).copy(),
        nl=np.array([[nl_gamma[0], nl_beta[0]]], np.float32),
        recip=recip.copy(),
        sb=sb, mgram=mgram,
        ident=np.eye(KD, dtype=np.float32),
        identb=np.eye(KD, dtype=np.float32).astype(__import__('ml_dtypes').bfloat16),
        ones=np.ones((C, 1), np.float32),
    )

    in_maps = []
    for c in range(N_CORES):
        fs = np.ascontiguousarray(features[:, :, c].reshape(B, C, V_SLICE))
        ms = np.ascontiguousarray(mask_features[:, :, c].reshape(B, C, V_SLICE))
        lab = init_masks[:, c].reshape(B, V_SLICE)
        labt = np.concatenate(
            [lab[b].reshape(V_SLICE // CHUNK, CHUNK).T for b in range(B)],
            axis=1).astype(np.float32)                  # [128, B*128]
        m = dict(feats=fs, mf=ms, labt=np.ascontiguousarray(labt))
        m.update(small)
        in_maps.append(m)

    nc = _get_nc()
    trace = os.environ.get("KERNEL_TRACE") == "1"
    res = bass_utils.run_bass_kernel_spmd(
        nc, in_maps, core_ids=list(range(N_CORES)), trace=trace)
    if trace:
        _compiled["exec_time_ns"] = res.exec_time_ns
        _compiled["mean_exec_time_ns"] = res.mean_exec_time_ns
    _compiled["last_results"] = res.results

    # ---- reassemble ----
    x = np.empty((B, C, D, H, W), np.float32)
    pred_masks = np.empty((B, NI, D, H, W), np.float32)
    for c in range(N_CORES):
        x[:, :, c] = res.results[c]["x_out"].reshape(B, C, H, W)
        pred_masks[:, :, c] = res.results[c]["masks_out"].reshape(B, NI, H, W)

    # inst_features / pred_kernel from the AllReduced segment sums (host math
    # mirrors the reference exactly)
    seg = res.results[0]["seg_out"]                      # [C, B*L]
    sums = seg.reshape(C, B, L).transpose(1, 2, 0)       # [B, L, C]
    means = sums / np.maximum(counts, 1.0)[:, :, None]
    means = means[:, 1:, :]                              # [B, 100, C]
    inst_features = np.concatenate(
        [means, np.broadcast_to(embed[None], (B, NLM, C))], axis=1)
    pred_kernel = inst_features @ mk_w.T + mk_b
    mu = pred_kernel.mean(axis=(0, 1))
    var = pred_kernel.var(axis=(0, 1))
    pred_kernel = (pred_kernel - mu) / np.sqrt(var + EPS) * nk_gamma + nk_beta

    return (x, inst_features.astype(np.float32), pred_masks,
            pred_kernel.astype(np.float32))


# revision 17
# speedup vs baseline: 1.0934x; 1.0934x over previous
"""Trainium2 Bass kernel for the ConnectedComponentsHead problem.

Sharding: 8 cores, core c owns depth slice d=c for both batches.
Per-core layout: batch 0 on SBUF partitions 0-63, batch 1 on partitions 64-127.

Pipeline (single SPMD launch):
  conv stack (4x conv3x3+bias+relu, float32r matmuls, PE quadrant pairing)
  -> segment sums per label (dma-transpose to [pixel, chan] bf16 tiles,
     one-hot matmul accumulation in PSUM)
  -> AllReduce segment sums across cores
  -> on-core pred_kernel algebra (segment means, mask_kernel linear, BN1d,
     BatchNorm2d-of-logits folded analytically to an affine a*x+c using
     host-precomputed Gram matrices of mask_features)
  -> bmm pred_kernel @ mask_features with fused +c epilogue.

Host (free) work: input slicing, label transposes/bincounts, Gram matrices,
inst_features/pred_kernel outputs from the AllReduced sums, reassembly.
"""

import os
import numpy as np

import concourse.bass as bass
from concourse import bacc, tile, bass_utils, mybir

F32 = mybir.dt.float32
F32R = mybir.dt.float32r
BF16 = mybir.dt.bfloat16
AF = mybir.ActivationFunctionType
OP = mybir.AluOpType
AX = mybir.AxisListType

N_CORES = 8
B, C, D, H, W = 2, 64, 8, 128, 128
L = 101
NLM = 20
KD = 64
NI = 100 + NLM          # 120 instances per batch
V_SLICE = H * W         # 16384 pixels per (b, d) slice
V = D * V_SLICE
EPS = 1e-3
BNV = float(B * 2 * NI * V) / 2.0  # B * N * V = 2*120*131072
INV_BNV = 1.0 / (B * NI * V)

HP = H + 2              # padded row count (130)
WP = W + 2              # padded col count (130)
TAPS = [(dy, dx) for dy in range(3) for dx in range(3)]
ROWS_PER_TILE = 3
N_TILES = (H + ROWS_PER_TILE - 1) // ROWS_PER_TILE  # 43
XB_PASS_T = [0, 11, 22, 33, N_TILES]       # conv3 tile ranges per xb pass
XB_COLS = 11 * ROWS_PER_TILE * W           # 4224 columns per xb pass buffer
CHUNK = 128             # pixels per segment chunk
MF_TILE = 512
N_MF_TILES = V_SLICE // MF_TILE  # 32

_compiled = {}


def _build_nc():
    nc = bacc.Bacc("TRN2", target_bir_lowering=False, debug=False,
                   num_devices=N_CORES)

    # ---- kernel I/O ----
    feats_in = nc.dram_tensor("feats", [B, C, V_SLICE], F32R, kind="ExternalInput")
    mf_in = nc.dram_tensor("mf", [B, C, V_SLICE], F32R, kind="ExternalInput")
    labt_in = nc.dram_tensor("labt", [CHUNK, B * (V_SLICE // CHUNK)], F32,
                             kind="ExternalInput")
    convw_in = nc.dram_tensor("convw", [128, 4 * 9 * 128], F32R, kind="ExternalInput")
    convb_in = nc.dram_tensor("convb", [128, 4], F32, kind="ExternalInput")
    mkwt_in = nc.dram_tensor("mkwt", [C, KD], F32, kind="ExternalInput")
    mkb_in = nc.dram_tensor("mkb", [KD, 1], F32, kind="ExternalInput")
    embt_in = nc.dram_tensor("embt", [C, NLM], F32, kind="ExternalInput")
    nkg_in = nc.dram_tensor("nkg", [KD, 1], F32, kind="ExternalInput")
    nkb_in = nc.dram_tensor("nkb", [KD, 1], F32, kind="ExternalInput")
    nl_in = nc.dram_tensor("nl", [1, 2], F32, kind="ExternalInput")  # [gamma, beta]
    recip_in = nc.dram_tensor("recip", [1, B * L], F32, kind="ExternalInput")
    sb_in = nc.dram_tensor("sb", [C, B], F32, kind="ExternalInput")
    mgram_in = nc.dram_tensor("mgram", [KD, B * KD], F32, kind="ExternalInput")
    ident_in = nc.dram_tensor("ident", [KD, KD], F32, kind="ExternalInput")
    identb_in = nc.dram_tensor("identb", [KD, KD], BF16, kind="ExternalInput")
    ones_in = nc.dram_tensor("ones", [C, 1], F32, kind="ExternalInput")

    x_out = nc.dram_tensor("x_out", [B, C, V_SLICE], F32, kind="ExternalOutput")
    masks_out = nc.dram_tensor("masks_out", [B, 2 * NI // 2, V_SLICE], F32,
                               kind="ExternalOutput")
    seg_out = nc.dram_tensor("seg_out", [C, B * L], F32, kind="ExternalOutput")

    with tile.TileContext(nc) as tc:
        with (
            tc.tile_pool(name="bufs", bufs=1) as bufpool,
            tc.tile_pool(name="const", bufs=1) as cpool,
            tc.tile_pool(name="work", bufs=4) as wpool,
            tc.tile_pool(name="alg", bufs=1) as apool,
            tc.tile_pool(name="apsum", bufs=4, space="PSUM") as apsum,
            tc.tile_pool(name="cpsum", bufs=2, space="PSUM") as cpsum,
            tc.tile_pool(name="spsum", bufs=1, space="PSUM") as spsum,
            tc.tile_pool(name="dram", bufs=1, space="DRAM") as dram,
        ):
            # ---- persistent SBUF tiles ----
            buf0 = bufpool.tile([128, HP, WP], F32R, tag="buf0")
            buf1 = bufpool.tile([128, HP, WP], F32R, tag="buf1")
            xb0 = bufpool.tile([64, XB_COLS], BF16, tag="xb0")
            xb1 = bufpool.tile([64, XB_COLS], BF16, tag="xb1")

            convw = cpool.tile([128, 4 * 9 * 128], F32R)
            convb = cpool.tile([128, 4], F32)
            labt = cpool.tile([CHUNK, B * (V_SLICE // CHUNK)], F32)
            iota = cpool.tile([CHUNK, L], BF16)
            mkwt = cpool.tile([C, KD], F32)
            mkb = cpool.tile([KD, 1], F32)
            embt = cpool.tile([C, NLM], F32)
            nkg = cpool.tile([KD, 1], F32)
            nkb = cpool.tile([KD, 1], F32)
            nlt = cpool.tile([1, 2], F32)
            recip = cpool.tile([1, B * L], F32)
            sbt = cpool.tile([C, B], F32)
            mgram = cpool.tile([KD, B * KD], F32)
            ident = cpool.tile([KD, KD], F32)
            identb = cpool.tile([KD, KD], BF16)
            ones = cpool.tile([C, 1], F32)

            nc.sync.dma_start(out=convw[:], in_=convw_in[:, :])
            nc.sync.dma_start(out=convb[:], in_=convb_in[:, :])
            nc.sync.dma_start(out=labt[:], in_=labt_in[:, :])
            nc.sync.dma_start(out=mkwt[:], in_=mkwt_in[:, :])
            nc.sync.dma_start(out=mkb[:], in_=mkb_in[:, :])
            nc.sync.dma_start(out=embt[:], in_=embt_in[:, :])
            nc.sync.dma_start(out=nkg[:], in_=nkg_in[:, :])
            nc.sync.dma_start(out=nkb[:], in_=nkb_in[:, :])
            nc.sync.dma_start(out=nlt[:], in_=nl_in[:, :])
            nc.sync.dma_start(out=recip[:], in_=recip_in[:, :])
            nc.sync.dma_start(out=sbt[:], in_=sb_in[:, :])
            nc.sync.dma_start(out=mgram[:], in_=mgram_in[:, :])
            nc.sync.dma_start(out=ident[:], in_=ident_in[:, :])
            nc.sync.dma_start(out=identb[:], in_=identb_in[:, :])
            nc.sync.dma_start(out=ones[:], in_=ones_in[:, :])
            nc.gpsimd.iota(iota[:], pattern=[[1, L]], base=0, channel_multiplier=0,
                           allow_small_or_imprecise_dtypes=True)

            # zero the padding halo of both conv buffers (never rewritten).
            # memset can't emit float32r, so copy from a zeroed fp32 row
            # (f32->f32r copy is a rounding op, which the BIR verifier wants).
            zrow = cpool.tile([128, WP], F32)
            nc.vector.memset(zrow[:], 0.0)
            zcol = zrow.rearrange("p (a b) -> p a b", b=1)
            for buf in (buf0, buf1):
                nc.vector.tensor_copy(buf[:, 0, :], zrow[:])
                nc.vector.tensor_copy(buf[:, HP - 1, :], zrow[:])
                nc.vector.tensor_copy(buf[:, :, 0:1], zcol)
                nc.vector.tensor_copy(buf[:, :, WP - 1:WP], zcol)

            # load conv0 input into the interior of buf0
            for b in range(B):
                nc.sync.dma_start(
                    out=buf0[64 * b:64 * b + 64, 1:H + 1, 1:W + 1],
                    in_=feats_in[b].rearrange("c (h w) -> c h w", w=W))

            # ---- conv stack ----
            bufs = [buf0, buf1]
            for layer in range(4):
                src = bufs[layer % 2]
                dst = bufs[(layer + 1) % 2]
                src_f = src.rearrange("p a b -> p (a b)")
                last = layer == 3
                for t in range(N_TILES):
                    r = ROWS_PER_TILE * t
                    nrows = min(ROWS_PER_TILE, H - r)
                    n = (nrows - 1) * WP + W
                    psA = apsum.tile([128, ROWS_PER_TILE, WP], F32, tag="psA")
                    psA_f = psA.rearrange("p a b -> p (a b)")
                    for k, (dy, dx) in enumerate(TAPS):
                        off = (r + dy) * WP + dx
                        wcol = (layer * 9 + k) * 128
                        nc.tensor.matmul(psA_f[:, 0:n],
                                         convw[:, wcol:wcol + 128],
                                         src_f[:, off:off + n],
                                         start=(k == 0), stop=(k == 8))
                    if not last:
                        nc.scalar.activation(
                            dst[:, r + 1:r + 1 + nrows, 1:W + 1],
                            psA[:, 0:nrows, 0:W],
                            AF.Relu, bias=convb[:, layer:layer + 1])
                    else:
                        # conv3: write x (f32r dst, DMA'd out via f32 bitcast)
                        # + bf16 copy for the segment path
                        nc.scalar.activation(
                            dst[:, r + 1:r + 1 + nrows, 1:W + 1],
                            psA[:, 0:nrows, 0:W],
                            AF.Relu, bias=convb[:, layer:layer + 1])
                        xpass = next(i for i in range(4)
                                     if XB_PASS_T[i] <= t < XB_PASS_T[i + 1])
                        xcol = (r - ROWS_PER_TILE * XB_PASS_T[xpass]) * W
                        npix = nrows * W
                        nc.vector.tensor_scalar(
                            xb0[:, xcol:xcol + npix],
                            psA[0:64, 0:nrows, 0:W],
                            convb[0:64, layer:layer + 1], 0.0, OP.add, OP.max)
                        nc.vector.tensor_scalar(
                            xb1[:, xcol:xcol + npix],
                            psA[64:128, 0:nrows, 0:W],
                            convb[64:128, layer:layer + 1], 0.0, OP.add, OP.max)
                        # segment matmuls for completed chunks of this pass
                        if t == XB_PASS_T[xpass + 1] - 1:
                            c_lo = XB_PASS_T[xpass] * ROWS_PER_TILE * W // CHUNK
                            c_hi = XB_PASS_T[xpass + 1] * ROWS_PER_TILE * W \
                                // CHUNK
                            c_hi = min(c_hi, V_SLICE // CHUNK)
                            base = XB_PASS_T[xpass] * ROWS_PER_TILE * W
                            for ch in range(c_lo, c_hi):
                                col = ch * CHUNK - base
                                for b in range(B):
                                    xbb = xb0 if b == 0 else xb1
                                    tp = cpsum.tile([CHUNK, C], BF16, tag="psB")
                                    nc.tensor.transpose(
                                        tp[:], xbb[:, col:col + CHUNK],
                                        identb[:])
                                    ft = wpool.tile([CHUNK, C], BF16, tag="ft")
                                    nc.scalar.activation(ft[:], tp[:], AF.Copy)
                                    oh = wpool.tile([CHUNK, L], BF16, tag="oh")
                                    nc.vector.tensor_scalar(
                                        oh[:], iota[:],
                                        labt[:, b * (V_SLICE // CHUNK) + ch:
                                             b * (V_SLICE // CHUNK) + ch + 1],
                                        None, OP.is_equal)
                                    segp = seg_ps0 if b == 0 else seg_ps1
                                    nc.tensor.matmul(
                                        segp[:], ft[:], oh[:],
                                        start=(ch == 0),
                                        stop=(ch == V_SLICE // CHUNK - 1))
                # allocate seg accumulators before conv3's tile loop runs
                if layer == 2:
                    seg_ps0 = spsum.tile([C, L], F32, tag="seg0")
                    seg_ps1 = spsum.tile([C, L], F32, tag="seg1")

            # x output DMA (conv3 wrote into buf0)
            for b in range(B):
                nc.sync.dma_start(
                    out=x_out[b].rearrange("c (h w) -> c h w", w=W),
                    in_=buf0[64 * b:64 * b + 64, 1:H + 1, 1:W + 1].bitcast(F32))

            # ---- segment sums AllReduce ----
            seg_sb = apool.tile([C, B * L], F32)
            nc.scalar.activation(seg_sb[:, 0:L], seg_ps0[:], AF.Copy)
            nc.scalar.activation(seg_sb[:, L:2 * L], seg_ps1[:], AF.Copy)
            cc_in = dram.tile([C, B * L], F32)
            cc_out = dram.tile([C, B * L], F32)
            nc.sync.dma_start(out=cc_in[:], in_=seg_sb[:])
            nc.gpsimd.collective_compute(
                "AllReduce", OP.add,
                replica_groups=[list(range(N_CORES))],
                ins=[cc_in.opt()], outs=[cc_out.opt()])
            sums = apool.tile([C, B * L], F32)
            nc.sync.dma_start(out=sums[:], in_=cc_out[:])
            nc.sync.dma_start(out=seg_out[:, :], in_=sums[:])

            # ---- pred_kernel algebra (identical on every core) ----
            recip_b = apool.tile([C, B * L], F32)
            nc.gpsimd.partition_broadcast(recip_b[:], recip[:])
            means = apool.tile([C, B * L], F32)
            nc.vector.tensor_tensor(means[:], sums[:], recip_b[:], OP.mult)

            instT = apool.tile([C, B * NI], F32)
            nc.vector.tensor_copy(instT[:, 0:100], means[:, 1:101])
            nc.vector.tensor_copy(instT[:, 100:120], embt[:])
            nc.vector.tensor_copy(instT[:, 120:220], means[:, L + 1:L + 101])
            nc.vector.tensor_copy(instT[:, 220:240], embt[:])

            pk_ps = cpsum.tile([KD, B * NI], F32, tag="psB")
            nc.tensor.matmul(pk_ps[:], mkwt[:], instT[:], start=True, stop=True)
            pk_sb = apool.tile([KD, B * NI], F32)
            nc.scalar.activation(pk_sb[:], pk_ps[:], AF.Identity, bias=mkb[:])

            red = apool.tile([KD, 1], F32)
            nc.vector.tensor_reduce(red[:], pk_sb[:], axis=AX.X, op=OP.add)
            mu = apool.tile([KD, 1], F32)
            nc.vector.tensor_scalar(mu[:], red[:], 1.0 / (B * NI), None, OP.mult)
            xc = apool.tile([KD, B * NI], F32)
            nc.vector.tensor_scalar(xc[:], pk_sb[:], mu[:], None, OP.subtract)
            sqs = apool.tile([KD, B * NI], F32)
            ssum = apool.tile([KD, 1], F32)
            nc.scalar.activation(sqs[:], xc[:], AF.Square, accum_out=ssum[:])
            varp = apool.tile([KD, 1], F32)
            nc.vector.tensor_scalar(varp[:], ssum[:], 1.0 / (B * NI), EPS,
                                    OP.mult, OP.add)
            std = apool.tile([KD, 1], F32)
            nc.scalar.activation(std[:], varp[:], AF.Sqrt)
            inv = apool.tile([KD, 1], F32)
            nc.vector.reciprocal(inv[:], std[:])
            sc = apool.tile([KD, 1], F32)
            nc.vector.tensor_tensor(sc[:], inv[:], nkg[:], OP.mult)
            pk_bnT = apool.tile([KD, B * NI], F32)
            nc.vector.tensor_scalar(pk_bnT[:], xc[:], sc[:], nkb[:],
                                    OP.mult, OP.add)

            # mu2 = sum_b s_b . (sum_n pk_bn) / BNV
            tsum = apool.tile([KD, 1], F32)
            pks0 = apool.tile([KD, 1], F32)
            pks1 = apool.tile([KD, 1], F32)
            nc.vector.tensor_reduce(pks0[:], pk_bnT[:, 0:NI], axis=AX.X, op=OP.add)
            nc.vector.tensor_reduce(pks1[:], pk_bnT[:, NI:2 * NI], axis=AX.X,
                                    op=OP.add)
            nc.vector.tensor_tensor(pks0[:], pks0[:], sbt[:, 0:1], OP.mult)
            nc.vector.tensor_tensor(pks1[:], pks1[:], sbt[:, 1:2], OP.mult)
            nc.vector.tensor_tensor(tsum[:], pks0[:], pks1[:], OP.add)
            sc_ps = cpsum.tile([1, 2], F32, tag="psB")
            nc.tensor.matmul(sc_ps[:, 0:1], ones[:], tsum[:], start=True, stop=True)
            mu2 = apool.tile([1, 1], F32)
            nc.scalar.activation(mu2[:], sc_ps[:, 0:1], AF.Copy, scale=INV_BNV)

            # E[x^2] = sum_b <G_b, M_b> / BNV  via PE transpose + Gram matmul
            gsum = apool.tile([KD, 1], F32)
            for b in range(B):
                tp_ps = cpsum.tile([NI, KD], F32, tag="psB")
                nc.tensor.transpose(tp_ps[:], pk_bnT[:, b * NI:(b + 1) * NI],
                                    ident[:])
                pkb = apool.tile([NI, KD], F32, tag=f"pkb{b}")
                nc.scalar.activation(pkb[:], tp_ps[:], AF.Copy)
                g_ps = cpsum.tile([KD, KD], F32, tag="psB")
                nc.tensor.matmul(g_ps[:], pkb[:], pkb[:], start=True, stop=True)
                ghm = apool.tile([KD, KD], F32, tag=f"ghm{b}")
                nc.vector.tensor_tensor(ghm[:], g_ps[:],
                                        mgram[:, b * KD:(b + 1) * KD], OP.mult)
                gr = apool.tile([KD, 1], F32, tag=f"gr{b}")
                nc.vector.tensor_reduce(gr[:], ghm[:], axis=AX.X, op=OP.add)
                if b == 0:
                    nc.vector.tensor_copy(gsum[:], gr[:])
                else:
                    nc.vector.tensor_tensor(gsum[:], gsum[:], gr[:], OP.add)
            e2_ps = cpsum.tile([1, 2], F32, tag="psB")
            nc.tensor.matmul(e2_ps[:, 0:1], ones[:], gsum[:], start=True, stop=True)
            e2 = apool.tile([1, 1], F32)
            nc.scalar.activation(e2[:], e2_ps[:, 0:1], AF.Copy, scale=INV_BNV)

            # a = nl_gamma / sqrt(var2 + eps); c = nl_beta - mu2 * a
            m2sq = apool.tile([1, 1], F32)
            nc.vector.tensor_tensor(m2sq[:], mu2[:], mu2[:], OP.mult)
            var2 = apool.tile([1, 1], F32)
            nc.vector.tensor_tensor(var2[:], e2[:], m2sq[:], OP.subtract)
            nc.vector.tensor_scalar(var2[:], var2[:], EPS, None, OP.add)
            std2 = apool.tile([1, 1], F32)
            nc.scalar.activation(std2[:], var2[:], AF.Sqrt)
            inv2 = apool.tile([1, 1], F32)
            nc.vector.reciprocal(inv2[:], std2[:])
            a_t = apool.tile([1, 1], F32)
            nc.vector.tensor_tensor(a_t[:], inv2[:], nlt[:, 0:1], OP.mult)
            na = apool.tile([1, 1], F32)
            nc.vector.tensor_scalar(na[:], a_t[:], -1.0, None, OP.mult)
            c_t = apool.tile([1, 1], F32)
            nc.vector.scalar_tensor_tensor(c_t[:], mu2[:], na[:], nlt[:, 1:2],
                                           OP.mult, OP.add)
            a_b = apool.tile([128, 1], F32)
            nc.gpsimd.partition_broadcast(a_b[:], a_t[:])
            c_b = apool.tile([128, 1], F32)
            nc.gpsimd.partition_broadcast(c_b[:], c_t[:])

            pk_sT = apool.tile([KD, B * NI], F32R)
            nc.vector.tensor_scalar(pk_sT[:], pk_bnT[:], a_b[0:KD, :], None,
                                    OP.mult)

            # ---- bmm: masks[b] = a*pk_bn[b] @ mf[b] + c ----
            for b in range(B):
                for t in range(N_MF_TILES):
                    mft = wpool.tile([C, MF_TILE], F32R, tag="mf")
                    nc.sync.dma_start(
                        out=mft[:],
                        in_=mf_in[b, :, t * MF_TILE:(t + 1) * MF_TILE])
                    bps = cpsum.tile([NI, MF_TILE], F32, tag="psB")
                    nc.tensor.matmul(bps[:], pk_sT[:, b * NI:(b + 1) * NI],
                                     mft[:], start=True, stop=True)
                    mo = wpool.tile([NI, MF_TILE], F32, tag="mo")
                    nc.scalar.activation(mo[:], bps[:], AF.Identity,
                                         bias=c_b[0:NI, :])
                    nc.sync.dma_start(
                        out=masks_out[b, :, t * MF_TILE:(t + 1) * MF_TILE],
                        in_=mo[:])

    nc.compile()
    return nc


def _get_nc():
    if "nc" not in _compiled:
        _compiled["nc"] = _build_nc()
    return _compiled["nc"]


def kernel(features, mask_features, conv_w, conv_b, embed, mk_w, mk_b,
           nk_gamma, nk_beta, nl_gamma, nl_beta, init_masks):
    features = np.asarray(features, np.float32)
    mask_features = np.asarray(mask_features, np.float32)
    conv_w = np.asarray(conv_w, np.float32)
    conv_b = np.asarray(conv_b, np.float32)
    embed = np.asarray(embed, np.float32)
    mk_w = np.asarray(mk_w, np.float32)
    mk_b = np.asarray(mk_b, np.float32)
    nk_gamma = np.asarray(nk_gamma, np.float32)
    nk_beta = np.asarray(nk_beta, np.float32)
    nl_gamma = np.asarray(nl_gamma, np.float32)
    nl_beta = np.asarray(nl_beta, np.float32)
    init_masks = np.asarray(init_masks, np.int32)

    # ---- host precompute (shared across cores) ----
    # conv weights: per (layer, tap) a [128,128] block-diag lhsT
    # diag(W.T, W.T) so one K=128 matmul computes both batch slices
    convw = np.zeros((128, 4 * 9 * 128), np.float32)
    for i in range(4):
        for k, (dy, dx) in enumerate(TAPS):
            wT = conv_w[i, :, :, 0, dy, dx].T           # [Cin, Cout]
            c0 = (i * 9 + k) * 128
            convw[0:64, c0:c0 + 64] = wT
            convw[64:128, c0 + 64:c0 + 128] = wT
    convb = np.ascontiguousarray(np.tile(conv_b.T, (2, 1)))    # [128, 4]

    counts = np.stack([np.bincount(init_masks[b].reshape(-1), minlength=L)
                       for b in range(B)]).astype(np.float32)   # [B, L]
    recip = (1.0 / np.maximum(counts, 1.0)).reshape(1, B * L)

    mfv = mask_features.reshape(B, C, V)
    sb = np.ascontiguousarray(mfv.sum(axis=2).T)                # [C, B]
    mgram = np.einsum('bkv,bjv->bkj', mfv, mfv,
                      optimize=True).astype(np.float32)
    mgram = np.ascontiguousarray(mgram.transpose(1, 0, 2).reshape(KD, B * KD))

    small = dict(
        convw=convw, convb=convb,
        mkwt=np.ascontiguousarray(mk_w.T),
        mkb=mk_b.reshape(KD, 1).copy(),
        embt=np.ascontiguousarray(embed.T),
        nkg=nk_gamma.reshape(KD, 1).copy(),
        nkb=nk_beta.reshape(KD, 1).copy(),
        nl=np.array([[nl_gamma[0], nl_beta[0]]], np.float32),
        recip=recip.copy(),
        sb=sb, mgram=mgram,
        ident=np.eye(KD, dtype=np.float32),
        identb=np.eye(KD, dtype=np.float32).astype(__import__('ml_dtypes').bfloat16),
        ones=np.ones((C, 1), np.float32),
    )

    in_maps = []
    for c in range(N_CORES):
        fs = np.ascontiguousarray(features[:, :, c].reshape(B, C, V_SLICE))
        ms = np.ascontiguousarray(mask_features[:, :, c].reshape(B, C, V_SLICE))
        lab = init_masks[:, c].reshape(B, V_SLICE)
        labt = np.concatenate(
            [lab[b].reshape(V_SLICE // CHUNK, CHUNK).T for b in range(B)],
            axis=1).astype(np.float32)                  # [128, B*128]
        m = dict(feats=fs, mf=ms, labt=np.ascontiguousarray(labt))
        m.update(small)
        in_maps.append(m)

    nc = _get_nc()
    trace = os.environ.get("KERNEL_TRACE") == "1"
    res = bass_utils.run_bass_kernel_spmd(
        nc, in_maps, core_ids=list(range(N_CORES)), trace=trace)
    if trace:
        _compiled["exec_time_ns"] = res.exec_time_ns
        _compiled["mean_exec_time_ns"] = res.mean_exec_time_ns
    _compiled["last_results"] = res.results

    # ---- reassemble ----
    x = np.empty((B, C, D, H, W), np.float32)
    pred_masks = np.empty((B, NI, D, H, W), np.float32)
    for c in range(N_CORES):
        x[:, :, c] = res.results[c]["x_out"].reshape(B, C, H, W)
        pred_masks[:, :, c] = res.results[c]["masks_out"].reshape(B, NI, H, W)

    # inst_features / pred_kernel from the AllReduced segment sums (host math
    # mirrors the reference exactly)
    seg = res.results[0]["seg_out"]                      # [C, B*L]
    sums = seg.reshape(C, B, L).transpose(1, 2, 0)       # [B, L, C]
    means = sums / np.maximum(counts, 1.0)[:, :, None]
    means = means[:, 1:, :]                              # [B, 100, C]
    inst_features = np.concatenate(
        [means, np.broadcast_to(embed[None], (B, NLM, C))], axis=1)
    pred_kernel = inst_features @ mk_w.T + mk_b
    mu = pred_kernel.mean(axis=(0, 1))
    var = pred_kernel.var(axis=(0, 1))
    pred_kernel = (pred_kernel - mu) / np.sqrt(var + EPS) * nk_gamma + nk_beta

    return (x, inst_features.astype(np.float32), pred_masks,
            pred_kernel.astype(np.float32))
